# revision 8
# baseline (speedup 1.0000x reference)
"""Trainium2 Bass kernel for nn_AlignGrapher (8 NeuronCores, SPMD). v2.

Restructurings (validated in numpy vs reference to ~3.3e-3 rel):
 - c_aggregation's sequential smoothing == fixed linear operator L [256,256],
   precomputed on host; fc1 folded into the patchify-conv -> Wf [4096,4096].
 - e-rows / pos-cols pruned to the 3584x224 entries the final crop uses.
 - conv + L as split-bf16 matmuls: W=Whi+Wlo, P=Phi+Plo (bf16), t1 =
   Whi@Phi + Whi@Plo + Wlo@Phi in one fp32 PSUM accumulation (error ~1e-5,
   75%% of fp32 PE cost); L-mix fp32.
 - KNN top-9: sim fp32 via split-bf16 q/zu; column index packed into the low
   9 bits of sim (14 mantissa bits kept): packed = (sim & 0xFFFFFE00) | col>>3.
   8 stride-8 chunk max8s -> 64 candidates; merge max8 + match_replace +
   reduce-max for the 9th; positions recovered with FindIndex8 over the 64
   candidates; orig col = (val & 0x1FF)*8 + (pos & 7).
 - EdgeConv: h = relu(U + max_k V[idx_k]), U = (A-B)q + gc_b fp32,
   V = Bw z fp32; single 9-row indirect DMA gather per 128-query tile.
 - One fp32 pair collective carries bottom-half queries + z (keys); both
   cores of a pair rebuild zu / Vt locally from the shared z.

Sharding: phase 1 data-parallel over 8 (branch,batch) units; phase 2 the
core pair (b, 4+b) splits batch b's 12544 query rows in half.
"""
import os
import numpy as np
import ml_dtypes

import concourse.bass as bass
import concourse.bacc as bacc_mod
import concourse.mybir as mybir
from concourse.tile import TileContext

C = 64
P = 8
IMG = 112
KNN = 9
E = 4096
NPOS = 256
PN = 14
N = IMG * IMG     # 12544
M = 3136          # 56*56
HALF = N // 2     # 6272
NT = HALF // 128  # 49 query tiles per core
EP = 3584         # pruned e-rows (qq' in [4,60))
SP = 224          # pruned pos cols (sc in [8,120), t in {0,1})
BN_EPS = 1e-5
NCORES = 8

F32 = mybir.dt.float32
BF16 = mybir.dt.bfloat16
U32 = mybir.dt.uint32
U8 = mybir.dt.uint8
NEG_BIG = -1.0e30
PHASE = os.environ.get('KERNEL_PHASE', 'full')

NPBF16 = ml_dtypes.bfloat16

# ----------------------------------------------------------------------------
# host-side constant prep
# ----------------------------------------------------------------------------

_E_KEEP = np.array([c * 64 + q for c in range(64) for q in range(4, 60)])
_S_KEEP = np.array([t * 128 + sc for t in range(2) for sc in range(8, 120)])


def _build_L():
    idxs = [i * PN + j for i in range(1, PN - 1) for j in range(1, PN)]
    offs = np.array([-PN, PN, -1, 1, -PN - 1, -PN + 1, PN - 1, PN + 1], np.int64)
    L = np.eye(NPOS, dtype=np.float64)
    for idx in idxs:
        L[idx, :] = L[idx + offs, :].mean(axis=0)
    return L


def _split16(x):
    hi = np.asarray(x, np.float32).astype(NPBF16)
    lo = (np.asarray(x, np.float32) - hi.astype(np.float32)).astype(NPBF16)
    return hi, lo


def _patchify(img):
    xp = np.zeros((C, IMG + 2 * P, IMG + 2 * P), dtype=np.float32)
    xp[:, P:IMG + P, P:IMG + P] = img
    return xp.reshape(C, 16, P, 16, P).transpose(0, 2, 4, 1, 3).reshape(E, NPOS).copy()


def _host_prep(inputs):
    L = _build_L()
    cagg_w = np.asarray(inputs['cagg_w'], np.float64)
    fc1_w = np.asarray(inputs['fc1_w'], np.float64)
    Wc4 = cagg_w.reshape(E, C * P * P).reshape(C, P * P, C * P * P)
    Wf = np.einsum('oc,cqk->oqk', fc1_w, Wc4).reshape(E, C * P * P)
    b4 = np.asarray(inputs['cagg_b'], np.float64).reshape(C, P * P)
    bfv = ((fc1_w @ b4).reshape(E)
           + np.repeat(np.asarray(inputs['fc1_b'], np.float64), P * P))

    Wp = Wf[_E_KEEP].astype(np.float32)                 # [3584, 4096]
    wft = np.ascontiguousarray(Wp.T)                    # [4096(k), 3584(e')]
    wfhi, wflo = _split16(wft)
    ltp = np.ascontiguousarray(
        L.T.astype(np.float32)[:, _S_KEEP])             # [256(q), 224(s)]
    bfp = bfv[_E_KEEP].astype(np.float32).reshape(EP, 1)

    gc_w = np.asarray(inputs['gc_w'], np.float32)
    A = gc_w[:, :C]; Bw = gc_w[:, C:]
    ambt = np.zeros((65, 128), NPBF16)
    ambt[:64, :] = (A - Bw).T
    ambt[64, :] = np.asarray(inputs['gc_b'], np.float32)
    bqt = Bw.T.copy()                                   # [64, 128]
    fc2wt = np.asarray(inputs['fc2_w'], np.float32).T.copy()   # [128, 64]

    bnp = np.zeros((64, 8), np.float32)
    bnp[:, 0] = inputs['bn1_g']; bnp[:, 1] = inputs['bn1_b']
    bnp[:, 2] = inputs['bn2_g']; bnp[:, 3] = inputs['bn2_b']
    bnp[:, 4] = inputs['fc2_b']

    iota8 = ((np.arange(M, dtype=np.uint32) >> np.uint32(4)) & 0xFF).reshape(1, M)
    uconst = np.zeros((128, 4), np.uint32)
    uconst[:, 0] = 0xFF             # j mask (8-bit intra-chunk iota)
    uconst[:, 1] = 4                # shift for j<<4
    uconst[:, 2] = 15               # and for pos&15
    uconst[:, 3] = 0xFFFFFF00       # pack mask

    return {
        'wfhi': wfhi, 'wflo': wflo, 'ltp': ltp, 'bfp': bfp,
        'ambt': ambt, 'bqt': bqt, 'fc2wt': fc2wt, 'bnp': bnp,
        'iota8': iota8, 'uconst': uconst,
        'ident': np.eye(128, dtype=np.float32),
    }


# ----------------------------------------------------------------------------
# device program
# ----------------------------------------------------------------------------

def build_program():
    nc = bacc_mod.Bacc('TRN2', target_bir_lowering=False, debug=False,
                       num_devices=NCORES)

    wfhi_d = nc.declare_dram_parameter('wfhi', [E, EP], BF16, isOutput=False)
    wflo_d = nc.declare_dram_parameter('wflo', [E, EP], BF16, isOutput=False)
    pmhi_d = nc.declare_dram_parameter('pmhi', [E, NPOS], BF16, isOutput=False)
    pmlo_d = nc.declare_dram_parameter('pmlo', [E, NPOS], BF16, isOutput=False)
    ltp_d = nc.declare_dram_parameter('ltp', [NPOS, SP], F32, isOutput=False)
    bfp_d = nc.declare_dram_parameter('bfp', [EP, 1], F32, isOutput=False)
    ambt_d = nc.declare_dram_parameter('ambt', [65, 128], BF16, isOutput=False)
    bqt_d = nc.declare_dram_parameter('bqt', [64, 128], F32, isOutput=False)
    fc2wt_d = nc.declare_dram_parameter('fc2wt', [128, 64], F32, isOutput=False)
    bnp_d = nc.declare_dram_parameter('bnp', [64, 8], F32, isOutput=False)
    iota8_d = nc.declare_dram_parameter('iota8', [1, M], U32, isOutput=False)
    uconst_d = nc.declare_dram_parameter('uconst', [128, 4], U32, isOutput=False)
    ident_d = nc.declare_dram_parameter('ident', [128, 128], F32, isOutput=False)
    masks_d = nc.declare_dram_parameter('masks', [128, 2], F32, isOutput=False)
    out_d = nc.declare_dram_parameter('out_half', [64, HALF], F32, isOutput=True)

    AL = mybir.AluOpType
    AF = mybir.ActivationFunctionType
    AX = mybir.AxisListType
    GRP_ALL = [list(range(NCORES))]
    GRP_PAIR = [[b, b + 4] for b in range(4)]

    with TileContext(nc) as tc:
        with tc.tile_pool(name='dram', bufs=1, space='DRAM') as dram, \
             tc.tile_pool(name='const', bufs=1) as cpool, \
             tc.tile_pool(name='persist', bufs=1) as ppool:

            t2d = dram.tile([EP, SP], F32, tag='t2d')
            cc1_in = dram.tile([64, 4], F32, tag='cc1i')
            cc1_out = dram.tile([64, 4], F32, tag='cc1o')
            cc2_in = dram.tile([64, HALF + M], F32, tag='cc2i')
            cc2_out = dram.tile([64, HALF + M], F32, tag='cc2o')
            cc3_in = dram.tile([64, 2], F32, tag='cc3i')
            cc3_out = dram.tile([64, 2], F32, tag='cc3o')
            rn_d = dram.tile([1, M], F32, tag='rnd')
            vt_d = dram.tile([M, 128], F32, tag='vtd')

            # small constants packed into one tile (slots are 4KiB-padded)
            cst = cpool.tile([128, 1024], F32, tag='cst')
            lt_sb = cst[:, 0:448].rearrange('p (a s) -> p a s', a=2)
            ident_sb = cst[:, 448:576]
            ambt_sb = cst[:65, 576:640].bitcast(BF16)
            bqt_sb = cst[:64, 704:832]
            fc2wt_sb = cst[:, 832:896]
            bnp_sb = cst[:64, 896:904]
            masks_sb = cst[:, 904:906]
            bfs_sb = cst[:, 906:934]
            ones_sb = cst[:64, 934:935]
            ucon_sb = cst[:, 936:940].bitcast(U32)
            nc.sync.dma_start(out=lt_sb, in_=ltp_d.rearrange('(a p) s -> p a s', p=128))
            nc.sync.dma_start(out=ident_sb, in_=ident_d[:, :])
            nc.sync.dma_start(out=ambt_sb, in_=ambt_d[:, :])
            nc.sync.dma_start(out=bqt_sb, in_=bqt_d[:, :])
            nc.sync.dma_start(out=fc2wt_sb, in_=fc2wt_d[:, :])
            nc.sync.dma_start(out=bnp_sb, in_=bnp_d[:, :])
            nc.sync.dma_start(out=masks_sb, in_=masks_d[:, :])
            nc.sync.dma_start(out=bfs_sb,
                              in_=bfp_d.rearrange('(et p) one -> p (et one)', p=128))
            nc.sync.dma_start(out=ucon_sb, in_=uconst_d[:, :])
            nc.vector.memset(ones_sb, 1.0)

            iota_sb = cpool.tile([128, M], U32, tag='iota')
            nc.sync.dma_start(out=iota_sb, in_=iota8_d[:, :].to_broadcast([128, M]))

            mask8 = ucon_sb[:, 0:1]
            c4u = ucon_sb[:, 1:2]
            c15u = ucon_sb[:, 2:3]
            mask24 = ucon_sb[:, 3:4]

            gate_sb = cpool.tile([128, 128], F32, tag='gate')
            gpsc = [0]

            def pe_gate(src_ap, pool=None, tag='g'):
                # Absorb pending deps of src_ap's producers into the PE clock.
                nc.scalar.copy(out=gate_sb[:src_ap.shape[0], :src_ap.shape[-1]],
                               in_=src_ap)
                if pool is None:
                    with tc.tile_pool(name=f'gps{gpsc[0]}', bufs=1,
                                      space='PSUM') as gps:
                        gp = gps.tile([128, 128], F32, tag='g',
                                      name=f'gp{gpsc[0]}')
                        nc.tensor.matmul(gp, lhsT=gate_sb, rhs=gate_sb,
                                         start=True, stop=True)
                else:
                    gp = pool.tile([128, 128], F32, tag=tag, name=f'gp{gpsc[0]}')
                    nc.tensor.matmul(gp, lhsT=gate_sb, rhs=gate_sb,
                                     start=True, stop=True)
                gpsc[0] += 1

            mx64 = masks_sb[:64, 0:1]
            my64 = masks_sb[:64, 1:2]

            # persistent across phase 2 (U uses split-bf16 q with ones row)
            qhi_sb = ppool.tile([65, HALF], BF16, tag='qhi')
            qlo_sb = ppool.tile([65, HALF], BF16, tag='qlo')
            zuhi_sb = ppool.tile([64, M], BF16, tag='zuhi')
            zulo_sb = ppool.tile([64, M], BF16, tag='zulo')
            outpre_sb = ppool.tile([64, HALF], F32, tag='outpre')

            # ---------------- phase 1a: split-bf16 conv --------------------
            ECW = (512, 512, 512, 256)   # e' chunk widths per eh half (1792)
            with tc.tile_pool(name='t1tp', bufs=1) as t1tp:
                t1t_sb = t1tp.tile([128, 2, EP], F32, tag='t1t')   # [:, qc, e']
                with tc.tile_pool(name='pm', bufs=1) as pmp, \
                     tc.tile_pool(name='wstream', bufs=3) as wsp, \
                     tc.tile_pool(name='ps1', bufs=8, space='PSUM') as ps1:
                    pmhi_sb = pmp.tile([128, 32, NPOS], BF16, tag='pmhi')
                    pmlo_sb = pmp.tile([128, 32, NPOS], BF16, tag='pmlo')
                    nc.sync.dma_start(out=pmhi_sb,
                                      in_=pmhi_d.rearrange('(k p) s -> p k s', p=128))
                    nc.sync.dma_start(out=pmlo_sb,
                                      in_=pmlo_d.rearrange('(k p) s -> p k s', p=128))
                    # absorb const-DMA sems, then pmat
                    for gi, off in enumerate((0, 448, 576, 704, 832, 896,
                                              904, 906, 936)):
                        nc.scalar.copy(out=gate_sb[:, gi:gi + 1],
                                       in_=cst[:, off:off + 1])
                    pe_gate(cst[:, 0:128], pool=ps1, tag='acc')
                    for eh in range(2):
                        if eh == 1:
                            pe_gate(t1t_sb[:, 0, 0:128], pool=ps1, tag='acc')
                        psums = [ps1.tile([128, 512], F32, tag='acc',
                                          name=f'acc{eh}_{i}') for i in range(8)]
                        for k in range(32):
                            whi_t = wsp.tile([128, 1792], BF16, tag='whi')
                            wlo_t = wsp.tile([128, 1792], BF16, tag='wlo')
                            nc.sync.dma_start(
                                out=whi_t, in_=wfhi_d[k * 128:(k + 1) * 128,
                                                      eh * 1792:(eh + 1) * 1792])
                            nc.sync.dma_start(
                                out=wlo_t, in_=wflo_d[k * 128:(k + 1) * 128,
                                                      eh * 1792:(eh + 1) * 1792])
                            for qc in range(2):
                                for ec in range(4):
                                    w = ECW[ec]
                                    off = 512 * ec
                                    for ti, (lh, rh) in enumerate(
                                            ((pmhi_sb, whi_t), (pmhi_sb, wlo_t),
                                             (pmlo_sb, whi_t))):
                                        nc.tensor.matmul(
                                            psums[qc * 4 + ec][:, :w],
                                            lhsT=lh[:, k, qc * 128:(qc + 1) * 128],
                                            rhs=rh[:, off:off + w],
                                            start=(k == 0 and ti == 0),
                                            stop=(k == 31 and ti == 2))
                        for qc in range(2):
                            for ec in range(4):
                                w = ECW[ec]
                                off = eh * 1792 + 512 * ec
                                nc.scalar.copy(
                                    out=t1t_sb[:, qc, off:off + w],
                                    in_=psums[qc * 4 + ec][:, :w])

                # ------------- phase 1b: L mix + bias ----------------------
                with tc.tile_pool(name='t2p', bufs=1) as t2p, \
                     tc.tile_pool(name='ps2', bufs=4, space='PSUM') as ps2:
                    t2_sb = t2p.tile([128, 28, SP], F32, tag='t2')
                    for et in range(28):
                        ps = ps2.tile([128, SP], F32, tag='mm2')
                        for qc in range(2):
                            nc.tensor.matmul(ps,
                                             lhsT=t1t_sb[:, qc, et * 128:(et + 1) * 128],
                                             rhs=lt_sb[:, qc, :],
                                             start=(qc == 0), stop=(qc == 1))
                        nc.scalar.activation(out=t2_sb[:, et, :], in_=ps,
                                             func=AF.Identity,
                                             bias=bfs_sb[:, et:et + 1])
                    # bounce through DRAM to switch to channel-major layout
                    nc.sync.dma_start(out=t2d.rearrange('(et p) s -> p et s', p=128),
                                      in_=t2_sb)

            # f_sb: channel-major image rows [64, 112*112] (no crop needed)
            with tc.tile_pool(name='pz', bufs=1) as pz:
              stg = pz.tile([64, HALF + M], F32, tag='stg')
              with tc.tile_pool(name='fp', bufs=1) as fp:
                f_sb = fp.tile([64, N], F32, tag='f')
                nc.sync.dma_start(out=f_sb,
                                  in_=t2d.rearrange('(c q) s -> c (q s)', c=64))

                # ------------- phase 1c: BN1 stats + allreduce -------------
                with tc.tile_pool(name='bn1', bufs=1) as bnp1:
                    fv = f_sb.rearrange('c (r w) -> c r w', w=448)
                    recs = bnp1.tile([64, 28, 6], F32, tag='recs')
                    for i in range(28):
                        nc.vector.bn_stats(out=recs[:, i, :], in_=fv[:, i, :])
                    sc = bnp1.tile([64, 24], F32, tag='sc')
                    mv = sc[:, 0:2]
                    tmp = sc[:, 2:3]
                    ssum = sc[:, 3:4]
                    ssq = sc[:, 4:5]
                    su = sc[:, 5:6]
                    qu = sc[:, 6:7]
                    mean = sc[:, 7:8]
                    var = sc[:, 8:9]
                    alpha = sc[:, 9:10]
                    beta = sc[:, 10:11]
                    aqm = sc[:, 11:12]
                    bqm = sc[:, 12:13]
                    azm = sc[:, 13:14]
                    bzm = sc[:, 14:15]
                    stage = sc[:, 16:20]
                    nc.vector.bn_aggr(out=mv, in_=recs)
                    nc.vector.tensor_scalar(out=ssum, in0=mv[:, 0:1], scalar1=float(N),
                                            scalar2=None, op0=AL.mult)
                    nc.vector.tensor_tensor(out=tmp, in0=mv[:, 0:1], in1=mv[:, 0:1],
                                            op=AL.mult)
                    nc.vector.tensor_tensor(out=ssq, in0=mv[:, 1:2], in1=tmp, op=AL.add)
                    nc.vector.tensor_scalar(out=ssq, in0=ssq, scalar1=float(N),
                                            scalar2=None, op0=AL.mult)
                    nc.vector.tensor_scalar(out=stage[:, 0:1], in0=ssum, scalar1=mx64,
                                            scalar2=None, op0=AL.mult)
                    nc.vector.tensor_scalar(out=stage[:, 1:2], in0=ssq, scalar1=mx64,
                                            scalar2=None, op0=AL.mult)
                    nc.vector.tensor_scalar(out=stage[:, 2:3], in0=ssum, scalar1=my64,
                                            scalar2=None, op0=AL.mult)
                    nc.vector.tensor_scalar(out=stage[:, 3:4], in0=ssq, scalar1=my64,
                                            scalar2=None, op0=AL.mult)
                    nc.sync.dma_start(out=cc1_in[:, :], in_=stage)
                    nc.gpsimd.collective_compute(
                        'AllReduce', AL.add, replica_groups=GRP_ALL,
                        ins=[cc1_in.opt()], outs=[cc1_out.opt()])
                    red = bnp1.tile([64, 4], F32, tag='red')
                    nc.sync.dma_start(out=red, in_=cc1_out[:, :])
                    # su/qu: this core's branch stats
                    nc.vector.tensor_scalar(out=tmp, in0=red[:, 2:3], scalar1=my64,
                                            scalar2=None, op0=AL.mult)
                    nc.vector.scalar_tensor_tensor(out=su, in0=red[:, 0:1], scalar=mx64,
                                                   in1=tmp, op0=AL.mult, op1=AL.add)
                    nc.vector.tensor_scalar(out=tmp, in0=red[:, 3:4], scalar1=my64,
                                            scalar2=None, op0=AL.mult)
                    nc.vector.scalar_tensor_tensor(out=qu, in0=red[:, 1:2], scalar=mx64,
                                                   in1=tmp, op0=AL.mult, op1=AL.add)
                    ncnt = 1.0 / (4.0 * N)
                    nc.vector.tensor_scalar(out=mean, in0=su, scalar1=ncnt,
                                            scalar2=None, op0=AL.mult)
                    nc.vector.tensor_scalar(out=var, in0=qu, scalar1=ncnt,
                                            scalar2=None, op0=AL.mult)
                    nc.vector.tensor_tensor(out=tmp, in0=mean, in1=mean, op=AL.mult)
                    nc.vector.tensor_tensor(out=var, in0=var, in1=tmp, op=AL.subtract)
                    nc.vector.tensor_scalar(out=var, in0=var, scalar1=BN_EPS,
                                            scalar2=None, op0=AL.add)
                    nc.scalar.activation(out=var, in_=var, func=AF.Sqrt)
                    nc.vector.reciprocal(out=var, in_=var)
                    nc.vector.tensor_tensor(out=alpha, in0=var, in1=bnp_sb[:, 0:1],
                                            op=AL.mult)
                    nc.vector.tensor_tensor(out=tmp, in0=mean, in1=alpha, op=AL.mult)
                    nc.vector.tensor_tensor(out=beta, in0=bnp_sb[:, 1:2], in1=tmp,
                                            op=AL.subtract)
                    # masked affine params: queries (x role), z (y role)
                    nc.vector.tensor_tensor(out=aqm, in0=alpha, in1=mx64, op=AL.mult)
                    nc.vector.tensor_tensor(out=bqm, in0=beta, in1=mx64, op=AL.mult)
                    nc.vector.tensor_scalar(out=azm, in0=alpha, scalar1=0.25,
                                            scalar2=my64, op0=AL.mult, op1=AL.mult)
                    nc.vector.tensor_tensor(out=bzm, in0=beta, in1=my64, op=AL.mult)

                    # ------------- phase 1d: stage + exchange --------------
                    with tc.tile_pool(name='pzdummy', bufs=1) as _pzd:
                        # bottom-half queries, BN'd, masked by mx
                        nc.vector.tensor_scalar(out=stg[:, 0:HALF],
                                                in0=f_sb[:, HALF:],
                                                scalar1=aqm, scalar2=bqm,
                                                op0=AL.mult, op1=AL.add)
                        # 2x2 sum-pool of raw features (3 strided adds)
                        fv = f_sb.rearrange('c (r a w b) -> c r a w b', r=56, a=2, w=56)
                        zv = stg[:, HALF:].rearrange('c (r w) -> c r w', w=56)
                        nc.vector.tensor_tensor(out=zv, in0=fv[:, :, 0, :, 0],
                                                in1=fv[:, :, 0, :, 1], op=AL.add)
                        nc.vector.tensor_tensor(out=zv, in0=zv,
                                                in1=fv[:, :, 1, :, 0], op=AL.add)
                        nc.vector.tensor_tensor(out=zv, in0=zv,
                                                in1=fv[:, :, 1, :, 1], op=AL.add)
                        nc.vector.tensor_scalar(out=stg[:, HALF:], in0=stg[:, HALF:],
                                                scalar1=azm, scalar2=bzm,
                                                op0=AL.mult, op1=AL.add)
                        nc.sync.dma_start(out=cc2_in[:, :], in_=stg)
                        nc.gpsimd.collective_compute(
                            'AllReduce', AL.add, replica_groups=GRP_PAIR,
                            ins=[cc2_in.opt()], outs=[cc2_out.opt()])
                        # local top half BN'd+masked overwrites the sent staging
                        nc.vector.tensor_scalar(out=stg[:, 0:HALF],
                                                in0=f_sb[:, 0:HALF],
                                                scalar1=aqm, scalar2=bqm,
                                                op0=AL.mult, op1=AL.add)

              # ---------- phase 1e: q, zu, Vt, hi/lo prep (f_sb freed) ----
              with tc.tile_pool(name='pz2', bufs=1) as pz2, \
                     tc.tile_pool(name='psn', bufs=2, space='PSUM') as psn, \
                     tc.tile_pool(name='vtp', bufs=2) as vtp:
                        rq = pz2.tile([64, HALF], F32, tag='rq')
                        tmpf = pz2.tile([64, HALF], F32, tag='tmpf')
                        nc.sync.dma_start(out=rq, in_=cc2_out[:, 0:HALF])
                        z_all = stg[:, HALF:]
                        nc.sync.dma_start(out=z_all, in_=cc2_out[:, HALF:])
                        # q = mx*(local top) + my*(received) in place
                        nc.vector.scalar_tensor_tensor(
                            out=rq, in0=rq, scalar=my64,
                            in1=stg[:, 0:HALF], op0=AL.mult, op1=AL.add)
                        nc.vector.memset(qhi_sb[64:65, :], 1.0)
                        nc.vector.memset(qlo_sb[64:65, :], 0.0)
                        nc.scalar.copy(out=qhi_sb[0:64, :], in_=rq)
                        nc.scalar.copy(out=tmpf, in_=qhi_sb[0:64, :])
                        nc.vector.tensor_tensor(out=qlo_sb[0:64, :],
                                                in0=rq, in1=tmpf,
                                                op=AL.subtract)
                        # zu: normalize z columns
                        zsq = tmpf[:, 0:M]
                        nc.vector.tensor_tensor(out=zsq, in0=z_all, in1=z_all,
                                                op=AL.mult)
                        nsq = pz2.tile([1, M], F32, tag='nsq')
                        for j in range(7):
                            psq = psn.tile([1, 448], F32, tag='nrm')
                            nc.tensor.matmul(psq, lhsT=ones_sb,
                                             rhs=zsq[:, j * 448:(j + 1) * 448],
                                             start=True, stop=True)
                            nc.scalar.copy(out=nsq[:, j * 448:(j + 1) * 448],
                                           in_=psq)
                        nc.vector.reciprocal(out=nsq, in_=nsq)
                        nc.scalar.activation(out=nsq, in_=nsq, func=AF.Sqrt)
                        nc.sync.dma_start(out=rn_d[:, :], in_=nsq)
                        zu = tmpf[:, 0:M]
                        nc.sync.dma_start(out=zu, in_=rn_d[:, :].to_broadcast([64, M]))
                        nc.vector.tensor_tensor(out=zu, in0=z_all, in1=zu,
                                                op=AL.mult)
                        nc.scalar.copy(out=zuhi_sb, in_=zu)
                        zuf = rq[:, 0:M]
                        nc.scalar.copy(out=zuf, in_=zuhi_sb)
                        nc.vector.tensor_tensor(out=zulo_sb, in0=zu, in1=zuf,
                                                op=AL.subtract)
                        # Vt = z^T @ Bw^T  (both cores; identical)
                        for mc in range(25):
                            w = 128 if mc < 24 else 64
                            pv = psn.tile([128, 128], F32, tag='vt')
                            nc.tensor.matmul(pv[:w, :],
                                             lhsT=z_all[:, mc * 128:mc * 128 + w],
                                             rhs=bqt_sb, start=True, stop=True)
                            vt = vtp.tile([128, 128], F32, tag='vtsb')
                            nc.scalar.copy(out=vt[:w, :], in_=pv[:w, :])
                            nc.sync.dma_start(
                                out=vt_d[mc * 128:mc * 128 + w, :],
                                in_=vt[:w, :])

            if PHASE == '1':
                nc.sync.dma_start(out=out_d[:, :], in_=outpre_sb)

            # ---------------- phase 2: sim + top9 + edgeconv + fc2 ---------
            if PHASE != '1':
                pe_gate(zuhi_sb.bitcast(F32)[:, 0:64])
                with tc.tile_pool(name='psim', bufs=5, space='PSUM') as psim, \
                     tc.tile_pool(name='pmisc', bufs=3, space='PSUM') as pmisc, \
                     tc.tile_pool(name='simp', bufs=2) as simp, \
                     tc.tile_pool(name='selp', bufs=2) as selp, \
                     tc.tile_pool(name='vgp', bufs=2) as vgp, \
                     tc.tile_pool(name='edge', bufs=2) as edgep:
                    for t in range(NT):
                        # --- sim (split-bf16), fp32 PSUM, ACT evict ---
                        sim = simp.tile([128, M], F32, tag='sim')
                        for j in range(7):
                            psj = psim.tile([128, 448], F32, tag='s')
                            for ti, (lh, rh) in enumerate(
                                    ((qhi_sb, zuhi_sb), (qhi_sb, zulo_sb),
                                     (qlo_sb, zuhi_sb))):
                                nc.tensor.matmul(
                                    psj, lhsT=lh[0:64, t * 128:(t + 1) * 128],
                                    rhs=rh[:, j * 448:(j + 1) * 448],
                                    start=(ti == 0), stop=(ti == 2))
                            nc.scalar.copy(out=sim[:, j * 448:(j + 1) * 448],
                                           in_=psj)
                        # --- pack intra-chunk iota into the low byte (DVE) ---
                        simu = sim.bitcast(U32)
                        nc.vector.scalar_tensor_tensor(
                            out=simu, in0=simu, scalar=mask24, in1=iota_sb,
                            op0=AL.bitwise_and, op1=AL.bitwise_or)
                        # --- 16 chunk max8s -> cand128 (transposed [r,k]) ---
                        sel = selp.tile([128, 144], F32, tag='sel')
                        cand = sel[:, 0:128]
                        candt = cand.rearrange('p (r k) -> p r k', k=16)
                        top9v = sel[:, 128:137]
                        pkv = sim.rearrange('p (j k) -> p j k', k=16)
                        for kk in range(16):
                            nc.vector.max(out=candt[:, :, kk], in_=pkv[:, :, kk])
                        # --- merge: top8, replace, 9th ---
                        nc.vector.max(out=top9v[:, 0:8], in_=cand)
                        candz = selp.tile([128, 128], F32, tag='cz')
                        nc.vector.match_replace(out=candz,
                                                in_to_replace=top9v[:, 0:8],
                                                in_values=cand, imm_value=NEG_BIG)
                        nc.vector.tensor_reduce(out=top9v[:, 8:9], in_=candz,
                                                axis=AX.X, op=AL.max)
                        # --- positions: two overlapping FindIndex8 ---
                        idxt = selp.tile([128, 32], F32, tag='idxt')
                        nc.vector.max_index(out=idxt[:, 1:9].bitcast(U32),
                                            in_max=top9v[:, 1:9], in_values=cand)
                        nc.vector.max_index(out=idxt[:, 0:8].bitcast(U32),
                                            in_max=top9v[:, 0:8], in_values=cand)
                        # --- index math: gi = ((val & 0xFF) << 4) | (pos & 15) ---
                        jsh = idxt[:, 16:25].bitcast(U32)
                        gi = selp.tile([128, 9], U32, tag='gi')
                        nc.vector.tensor_scalar(out=jsh, in0=top9v.bitcast(U32),
                                                scalar1=mask8, scalar2=None,
                                                op0=AL.bitwise_and)
                        nc.vector.tensor_scalar(out=jsh, in0=jsh, scalar1=c4u,
                                                scalar2=None,
                                                op0=AL.logical_shift_left)
                        posall = idxt[:, 0:9].bitcast(U32)
                        nc.vector.tensor_scalar(out=gi, in0=posall, scalar1=c15u,
                                                scalar2=None, op0=AL.bitwise_and)
                        nc.vector.tensor_tensor(out=gi, in0=gi, in1=jsh,
                                                op=AL.bitwise_or)
                        # --- gather V rows (9 single-offset indirect DMAs) ---
                        vg = vgp.tile([128, KNN, 128], F32, tag='vg')
                        if PHASE == '2':
                            nc.vector.memset(vg, 0.0)
                        else:
                            for k in range(KNN):
                                nc.gpsimd.indirect_dma_start(
                                    out=vg[:, k, :], out_offset=None,
                                    in_=vt_d[:, :],
                                    in_offset=bass.IndirectOffsetOnAxis(
                                        ap=gi[:, k:k + 1], axis=0))
                        # --- V-max (DVE; Pool lacks max) ---
                        vmx = edgep.tile([128, 384], F32, tag='vmx')
                        va = vmx[:, 0:128]
                        vb = vmx[:, 128:256]
                        nc.vector.tensor_tensor(out=va, in0=vg[:, 0, :],
                                                in1=vg[:, 1, :], op=AL.max)
                        nc.vector.tensor_tensor(out=va, in0=va, in1=vg[:, 2, :],
                                                op=AL.max)
                        nc.vector.tensor_tensor(out=va, in0=va, in1=vg[:, 3, :],
                                                op=AL.max)
                        nc.vector.tensor_tensor(out=va, in0=va, in1=vg[:, 4, :],
                                                op=AL.max)
                        nc.vector.tensor_tensor(out=vb, in0=vg[:, 5, :],
                                                in1=vg[:, 6, :], op=AL.max)
                        nc.vector.tensor_tensor(out=vb, in0=vb, in1=vg[:, 7, :],
                                                op=AL.max)
                        nc.vector.tensor_tensor(out=vb, in0=vb, in1=vg[:, 8, :],
                                                op=AL.max)
                        # --- U + vmax ---
                        pu = pmisc.tile([128, 128], F32, tag='m', name=f'pu{t}')
                        nc.tensor.matmul(pu, lhsT=qhi_sb[:, t * 128:(t + 1) * 128],
                                         rhs=ambt_sb, start=True, stop=False)
                        nc.tensor.matmul(pu, lhsT=qlo_sb[:, t * 128:(t + 1) * 128],
                                         rhs=ambt_sb, start=False, stop=True)
                        usb = vmx[:, 256:384]
                        nc.scalar.copy(out=usb, in_=pu)
                        hts = edgep.tile([128, 256], F32, tag='hts')
                        ht = hts[:, 0:128]
                        hs = hts[:, 128:256]
                        nc.vector.tensor_tensor(out=ht, in0=va, in1=vb, op=AL.max)
                        nc.gpsimd.tensor_tensor(out=ht, in0=ht, in1=usb, op=AL.add)
                        # transpose, relu on evict, fc2
                        ph = pmisc.tile([128, 128], F32, tag='m', name=f'ph{t}')
                        nc.tensor.transpose(ph, ht, ident_sb)
                        nc.scalar.activation(out=hs, in_=ph, func=AF.Relu)
                        po = pmisc.tile([64, 128], F32, tag='m', name=f'po{t}')
                        nc.tensor.matmul(po, lhsT=fc2wt_sb, rhs=hs,
                                         start=True, stop=True)
                        nc.scalar.activation(out=outpre_sb[:, t * 128:(t + 1) * 128],
                                             in_=po, func=AF.Identity,
                                             bias=bnp_sb[:, 4:5])

                # ------------ phase 3: BN2 + output ------------------------
                with tc.tile_pool(name='bn2', bufs=1) as bnp2:
                    recs2 = bnp2.tile([64, 14, 6], F32, tag='recs2')
                    opv = outpre_sb.rearrange('c (a b) -> c a b', b=448)
                    for i in range(14):
                        nc.vector.bn_stats(out=recs2[:, i, :], in_=opv[:, i, :])
                    sc2 = bnp2.tile([64, 16], F32, tag='sc2')
                    mv2 = sc2[:, 0:2]
                    st2 = sc2[:, 2:4]
                    tmp2 = sc2[:, 4:5]
                    mean2 = sc2[:, 5:6]
                    var2 = sc2[:, 6:7]
                    a2 = sc2[:, 7:8]
                    b2 = sc2[:, 8:9]
                    nc.vector.bn_aggr(out=mv2, in_=recs2)
                    nc.vector.tensor_scalar(out=st2[:, 0:1], in0=mv2[:, 0:1],
                                            scalar1=float(HALF), scalar2=None,
                                            op0=AL.mult)
                    nc.vector.tensor_tensor(out=tmp2, in0=mv2[:, 0:1],
                                            in1=mv2[:, 0:1], op=AL.mult)
                    nc.vector.tensor_tensor(out=st2[:, 1:2], in0=mv2[:, 1:2],
                                            in1=tmp2, op=AL.add)
                    nc.vector.tensor_scalar(out=st2[:, 1:2], in0=st2[:, 1:2],
                                            scalar1=float(HALF), scalar2=None,
                                            op0=AL.mult)
                    nc.sync.dma_start(out=cc3_in[:, :], in_=st2)
                    nc.gpsimd.collective_compute(
                        'AllReduce', AL.add, replica_groups=GRP_ALL,
                        ins=[cc3_in.opt()], outs=[cc3_out.opt()])
                    red2 = bnp2.tile([64, 2], F32, tag='red2')
                    nc.sync.dma_start(out=red2, in_=cc3_out[:, :])
                    ncnt2 = 1.0 / (NCORES * HALF)
                    nc.vector.tensor_scalar(out=mean2, in0=red2[:, 0:1], scalar1=ncnt2,
                                            scalar2=None, op0=AL.mult)
                    nc.vector.tensor_scalar(out=var2, in0=red2[:, 1:2], scalar1=ncnt2,
                                            scalar2=None, op0=AL.mult)
                    nc.vector.tensor_tensor(out=tmp2, in0=mean2, in1=mean2, op=AL.mult)
                    nc.vector.tensor_tensor(out=var2, in0=var2, in1=tmp2,
                                            op=AL.subtract)
                    nc.vector.tensor_scalar(out=var2, in0=var2, scalar1=BN_EPS,
                                            scalar2=None, op0=AL.add)
                    nc.scalar.activation(out=var2, in_=var2, func=AF.Sqrt)
                    nc.vector.reciprocal(out=var2, in_=var2)
                    nc.vector.tensor_tensor(out=a2, in0=var2, in1=bnp_sb[:, 2:3],
                                            op=AL.mult)
                    nc.vector.tensor_tensor(out=tmp2, in0=mean2, in1=a2, op=AL.mult)
                    nc.vector.tensor_tensor(out=b2, in0=bnp_sb[:, 3:4], in1=tmp2,
                                            op=AL.subtract)
                    nc.vector.tensor_scalar(out=outpre_sb, in0=outpre_sb, scalar1=a2,
                                            scalar2=b2, op0=AL.mult, op1=AL.add)
                    nc.sync.dma_start(out=out_d[:, :], in_=outpre_sb)

    nc.compile()
    return nc


_PROGRAM = None


def _get_program():
    global _PROGRAM
    if _PROGRAM is None:
        _PROGRAM = build_program()
    return _PROGRAM


def make_inmaps(inputs):
    prep = _host_prep(inputs)
    x = np.asarray(inputs['x'], np.float32)
    y = np.asarray(inputs['y'], np.float32)
    in_maps = []
    for core in range(NCORES):
        img = x[core] if core < 4 else y[core - 4]
        mx, my = (1.0, 0.0) if core < 4 else (0.0, 1.0)
        masks = np.zeros((128, 2), np.float32)
        masks[:, 0] = mx
        masks[:, 1] = my
        pm = _patchify(img)
        pmhi, pmlo = _split16(pm)
        in_maps.append({
            'pmhi': pmhi, 'pmlo': pmlo,
            'masks': masks,
            **{k: prep[k] for k in ('wfhi', 'wflo', 'ltp', 'bfp', 'ambt', 'bqt',
                                    'fc2wt', 'bnp', 'iota8', 'uconst', 'ident')},
        })
    return in_maps


def assemble(results, inputs):
    x = np.asarray(inputs['x'], np.float32)
    y = np.asarray(inputs['y'], np.float32)
    out = np.empty((4, 64, N), np.float32)
    for b in range(4):
        out[b, :, :HALF] = results[b]['out_half']
        out[b, :, HALF:] = results[b + 4]['out_half']
    out = out.reshape(4, 64, IMG, IMG)
    return out + x, out + y


def kernel(**inputs):
    from concourse.bass_utils import run_bass_kernel_spmd
    nc = _get_program()
    in_maps = make_inmaps(inputs)
    res = run_bass_kernel_spmd(nc, in_maps, core_ids=list(range(NCORES)))
    return assemble(res.results, inputs)


# revision 9
# speedup vs baseline: 1.0212x; 1.0212x over previous
"""Trainium2 Bass kernel for nn_AlignGrapher (8 NeuronCores, SPMD). v2.

Restructurings (validated in numpy vs reference to ~3.3e-3 rel):
 - c_aggregation's sequential smoothing == fixed linear operator L [256,256],
   precomputed on host; fc1 folded into the patchify-conv -> Wf [4096,4096].
 - e-rows / pos-cols pruned to the 3584x224 entries the final crop uses.
 - conv + L as split-bf16 matmuls: W=Whi+Wlo, P=Phi+Plo (bf16), t1 =
   Whi@Phi + Whi@Plo + Wlo@Phi in one fp32 PSUM accumulation (error ~1e-5,
   75%% of fp32 PE cost); L-mix fp32.
 - KNN top-9: sim fp32 via split-bf16 q/zu; column index packed into the low
   9 bits of sim (14 mantissa bits kept): packed = (sim & 0xFFFFFE00) | col>>3.
   8 stride-8 chunk max8s -> 64 candidates; merge max8 + match_replace +
   reduce-max for the 9th; positions recovered with FindIndex8 over the 64
   candidates; orig col = (val & 0x1FF)*8 + (pos & 7).
 - EdgeConv: h = relu(U + max_k V[idx_k]), U = (A-B)q + gc_b fp32,
   V = Bw z fp32; single 9-row indirect DMA gather per 128-query tile.
 - One fp32 pair collective carries bottom-half queries + z (keys); both
   cores of a pair rebuild zu / Vt locally from the shared z.

Sharding: phase 1 data-parallel over 8 (branch,batch) units; phase 2 the
core pair (b, 4+b) splits batch b's 12544 query rows in half.
"""
import os
import numpy as np
import ml_dtypes

import concourse.bass as bass
import concourse.bacc as bacc_mod
import concourse.mybir as mybir
from concourse.tile import TileContext

C = 64
P = 8
IMG = 112
KNN = 9
E = 4096
NPOS = 256
PN = 14
N = IMG * IMG     # 12544
M = 3136          # 56*56
HALF = N // 2     # 6272
NT = HALF // 128  # 49 query tiles per core
EP = 3584         # pruned e-rows (qq' in [4,60))
SP = 224          # pruned pos cols (sc in [8,120), t in {0,1})
BN_EPS = 1e-5
NCORES = 8

F32 = mybir.dt.float32
BF16 = mybir.dt.bfloat16
U32 = mybir.dt.uint32
U8 = mybir.dt.uint8
NEG_BIG = -1.0e30
PHASE = os.environ.get('KERNEL_PHASE', 'full')

NPBF16 = ml_dtypes.bfloat16

# ----------------------------------------------------------------------------
# host-side constant prep
# ----------------------------------------------------------------------------

_E_KEEP = np.array([c * 64 + q for c in range(64) for q in range(4, 60)])
_S_KEEP = np.array([t * 128 + sc for t in range(2) for sc in range(8, 120)])


def _build_L():
    idxs = [i * PN + j for i in range(1, PN - 1) for j in range(1, PN)]
    offs = np.array([-PN, PN, -1, 1, -PN - 1, -PN + 1, PN - 1, PN + 1], np.int64)
    L = np.eye(NPOS, dtype=np.float64)
    for idx in idxs:
        L[idx, :] = L[idx + offs, :].mean(axis=0)
    return L


def _split16(x):
    hi = np.asarray(x, np.float32).astype(NPBF16)
    lo = (np.asarray(x, np.float32) - hi.astype(np.float32)).astype(NPBF16)
    return hi, lo


def _patchify(img):
    xp = np.zeros((C, IMG + 2 * P, IMG + 2 * P), dtype=np.float32)
    xp[:, P:IMG + P, P:IMG + P] = img
    return xp.reshape(C, 16, P, 16, P).transpose(0, 2, 4, 1, 3).reshape(E, NPOS).copy()


def _host_prep(inputs):
    L = _build_L()
    cagg_w = np.asarray(inputs['cagg_w'], np.float64)
    fc1_w = np.asarray(inputs['fc1_w'], np.float64)
    Wc4 = cagg_w.reshape(E, C * P * P).reshape(C, P * P, C * P * P)
    Wf = np.einsum('oc,cqk->oqk', fc1_w, Wc4).reshape(E, C * P * P)
    b4 = np.asarray(inputs['cagg_b'], np.float64).reshape(C, P * P)
    bfv = ((fc1_w @ b4).reshape(E)
           + np.repeat(np.asarray(inputs['fc1_b'], np.float64), P * P))

    Wp = Wf[_E_KEEP].astype(np.float32)                 # [3584, 4096]
    wft = np.ascontiguousarray(Wp.T)                    # [4096(k), 3584(e')]
    wfhi, wflo = _split16(wft)
    ltp = np.ascontiguousarray(
        L.T.astype(np.float32)[:, _S_KEEP])             # [256(q), 224(s)]
    bfp = bfv[_E_KEEP].astype(np.float32).reshape(EP, 1)

    gc_w = np.asarray(inputs['gc_w'], np.float32)
    A = gc_w[:, :C]; Bw = gc_w[:, C:]
    ambt = np.zeros((65, 128), NPBF16)
    ambt[:64, :] = (A - Bw).T
    ambt[64, :] = np.asarray(inputs['gc_b'], np.float32)
    bqt = Bw.T.copy()                                   # [64, 128]
    fc2wt = np.asarray(inputs['fc2_w'], np.float32).T.copy()   # [128, 64]

    bnp = np.zeros((64, 8), np.float32)
    bnp[:, 0] = inputs['bn1_g']; bnp[:, 1] = inputs['bn1_b']
    bnp[:, 2] = inputs['bn2_g']; bnp[:, 3] = inputs['bn2_b']
    bnp[:, 4] = inputs['fc2_b']

    iota8 = (np.arange(M, dtype=np.uint32) >> np.uint32(3)).reshape(1, M)
    uconst = np.zeros((128, 4), np.uint32)
    uconst[:, 0] = 0x1FF            # j mask (9-bit intra-chunk iota)
    uconst[:, 1] = 3                # shift for j<<3
    uconst[:, 2] = 7                # and for pos&7
    uconst[:, 3] = 0xFFFFFE00       # pack mask (14 mantissa bits)

    return {
        'wfhi': wfhi, 'wflo': wflo, 'ltp': ltp, 'bfp': bfp,
        'ambt': ambt, 'bqt': bqt, 'fc2wt': fc2wt, 'bnp': bnp,
        'iota8': iota8, 'uconst': uconst,
        'ident': np.eye(128, dtype=np.float32),
    }


# ----------------------------------------------------------------------------
# device program
# ----------------------------------------------------------------------------

def build_program():
    nc = bacc_mod.Bacc('TRN2', target_bir_lowering=False, debug=False,
                       num_devices=NCORES)

    wfhi_d = nc.declare_dram_parameter('wfhi', [E, EP], BF16, isOutput=False)
    wflo_d = nc.declare_dram_parameter('wflo', [E, EP], BF16, isOutput=False)
    pmhi_d = nc.declare_dram_parameter('pmhi', [E, NPOS], BF16, isOutput=False)
    pmlo_d = nc.declare_dram_parameter('pmlo', [E, NPOS], BF16, isOutput=False)
    ltp_d = nc.declare_dram_parameter('ltp', [NPOS, SP], F32, isOutput=False)
    bfp_d = nc.declare_dram_parameter('bfp', [EP, 1], F32, isOutput=False)
    ambt_d = nc.declare_dram_parameter('ambt', [65, 128], BF16, isOutput=False)
    bqt_d = nc.declare_dram_parameter('bqt', [64, 128], F32, isOutput=False)
    fc2wt_d = nc.declare_dram_parameter('fc2wt', [128, 64], F32, isOutput=False)
    bnp_d = nc.declare_dram_parameter('bnp', [64, 8], F32, isOutput=False)
    iota8_d = nc.declare_dram_parameter('iota8', [1, M], U32, isOutput=False)
    uconst_d = nc.declare_dram_parameter('uconst', [128, 4], U32, isOutput=False)
    ident_d = nc.declare_dram_parameter('ident', [128, 128], F32, isOutput=False)
    masks_d = nc.declare_dram_parameter('masks', [128, 2], F32, isOutput=False)
    out_d = nc.declare_dram_parameter('out_half', [64, HALF], F32, isOutput=True)

    AL = mybir.AluOpType
    AF = mybir.ActivationFunctionType
    AX = mybir.AxisListType
    GRP_ALL = [list(range(NCORES))]
    GRP_PAIR = [[b, b + 4] for b in range(4)]

    with TileContext(nc) as tc:
        with tc.tile_pool(name='dram', bufs=1, space='DRAM') as dram, \
             tc.tile_pool(name='const', bufs=1) as cpool, \
             tc.tile_pool(name='persist', bufs=1) as ppool:

            t2d = dram.tile([EP, SP], F32, tag='t2d')
            cc1_in = dram.tile([64, 4], F32, tag='cc1i')
            cc1_out = dram.tile([64, 4], F32, tag='cc1o')
            cc2_in = dram.tile([64, HALF + M], F32, tag='cc2i')
            cc2_out = dram.tile([64, HALF + M], F32, tag='cc2o')
            cc3_in = dram.tile([64, 2], F32, tag='cc3i')
            cc3_out = dram.tile([64, 2], F32, tag='cc3o')
            rn_d = dram.tile([1, M], F32, tag='rnd')
            vt_d = dram.tile([M, 128], BF16, tag='vtd')

            # small constants packed into one tile (slots are 4KiB-padded)
            cst = cpool.tile([128, 1024], F32, tag='cst')
            lt_sb = cst[:, 0:448].rearrange('p (a s) -> p a s', a=2)
            ident_sb = cst[:, 448:576]
            ambt_sb = cst[:65, 576:640].bitcast(BF16)
            bqt_sb = cst[:64, 704:832]
            fc2wt_sb = cst[:, 832:896]
            bnp_sb = cst[:64, 896:904]
            masks_sb = cst[:, 904:906]
            bfs_sb = cst[:, 906:934]
            ones_sb = cst[:64, 934:935]
            ucon_sb = cst[:, 936:940].bitcast(U32)
            nc.sync.dma_start(out=lt_sb, in_=ltp_d.rearrange('(a p) s -> p a s', p=128))
            nc.sync.dma_start(out=ident_sb, in_=ident_d[:, :])
            nc.sync.dma_start(out=ambt_sb, in_=ambt_d[:, :])
            nc.sync.dma_start(out=bqt_sb, in_=bqt_d[:, :])
            nc.sync.dma_start(out=fc2wt_sb, in_=fc2wt_d[:, :])
            nc.sync.dma_start(out=bnp_sb, in_=bnp_d[:, :])
            nc.sync.dma_start(out=masks_sb, in_=masks_d[:, :])
            nc.sync.dma_start(out=bfs_sb,
                              in_=bfp_d.rearrange('(et p) one -> p (et one)', p=128))
            nc.sync.dma_start(out=ucon_sb, in_=uconst_d[:, :])
            nc.vector.memset(ones_sb, 1.0)

            iota_sb = cpool.tile([128, M], U32, tag='iota')
            nc.sync.dma_start(out=iota_sb, in_=iota8_d[:, :].to_broadcast([128, M]))

            mask8 = ucon_sb[:, 0:1]
            c4u = ucon_sb[:, 1:2]
            c15u = ucon_sb[:, 2:3]
            mask24 = ucon_sb[:, 3:4]

            gate_sb = cpool.tile([128, 128], F32, tag='gate')
            gpsc = [0]

            def pe_gate(src_ap, pool=None, tag='g'):
                # Absorb pending deps of src_ap's producers into the PE clock.
                nc.scalar.copy(out=gate_sb[:src_ap.shape[0], :src_ap.shape[-1]],
                               in_=src_ap)
                if pool is None:
                    with tc.tile_pool(name=f'gps{gpsc[0]}', bufs=1,
                                      space='PSUM') as gps:
                        gp = gps.tile([128, 128], F32, tag='g',
                                      name=f'gp{gpsc[0]}')
                        nc.tensor.matmul(gp, lhsT=gate_sb, rhs=gate_sb,
                                         start=True, stop=True)
                else:
                    gp = pool.tile([128, 128], F32, tag=tag, name=f'gp{gpsc[0]}')
                    nc.tensor.matmul(gp, lhsT=gate_sb, rhs=gate_sb,
                                     start=True, stop=True)
                gpsc[0] += 1

            mx64 = masks_sb[:64, 0:1]
            my64 = masks_sb[:64, 1:2]

            # persistent across phase 2 (U uses split-bf16 q with ones row)
            qhi_sb = ppool.tile([65, HALF], BF16, tag='qhi')
            qlo_sb = ppool.tile([65, HALF], BF16, tag='qlo')
            zuhi_sb = ppool.tile([64, M], BF16, tag='zuhi')
            zulo_sb = ppool.tile([64, M], BF16, tag='zulo')
            outpre_sb = ppool.tile([64, HALF], F32, tag='outpre')

            # ---------------- phase 1a: split-bf16 conv --------------------
            ECW = (512, 512, 512, 256)   # e' chunk widths per eh half (1792)
            with tc.tile_pool(name='t1tp', bufs=1) as t1tp:
                t1t_sb = t1tp.tile([128, 2, EP], F32, tag='t1t')   # [:, qc, e']
                with tc.tile_pool(name='pm', bufs=1) as pmp, \
                     tc.tile_pool(name='wstream', bufs=3) as wsp, \
                     tc.tile_pool(name='ps1', bufs=8, space='PSUM') as ps1:
                    pmhi_sb = pmp.tile([128, 32, NPOS], BF16, tag='pmhi')
                    pmlo_sb = pmp.tile([128, 32, NPOS], BF16, tag='pmlo')
                    nc.sync.dma_start(out=pmhi_sb,
                                      in_=pmhi_d.rearrange('(k p) s -> p k s', p=128))
                    nc.sync.dma_start(out=pmlo_sb,
                                      in_=pmlo_d.rearrange('(k p) s -> p k s', p=128))
                    # absorb const-DMA sems, then pmat
                    for gi, off in enumerate((0, 448, 576, 704, 832, 896,
                                              904, 906, 936)):
                        nc.scalar.copy(out=gate_sb[:, gi:gi + 1],
                                       in_=cst[:, off:off + 1])
                    pe_gate(cst[:, 0:128], pool=ps1, tag='acc')
                    for eh in range(2):
                        if eh == 1:
                            pe_gate(t1t_sb[:, 0, 0:128], pool=ps1, tag='acc')
                        psums = [ps1.tile([128, 512], F32, tag='acc',
                                          name=f'acc{eh}_{i}') for i in range(8)]
                        for k in range(32):
                            whi_t = wsp.tile([128, 1792], BF16, tag='whi')
                            wlo_t = wsp.tile([128, 1792], BF16, tag='wlo')
                            nc.sync.dma_start(
                                out=whi_t, in_=wfhi_d[k * 128:(k + 1) * 128,
                                                      eh * 1792:(eh + 1) * 1792])
                            nc.sync.dma_start(
                                out=wlo_t, in_=wflo_d[k * 128:(k + 1) * 128,
                                                      eh * 1792:(eh + 1) * 1792])
                            for qc in range(2):
                                for ec in range(4):
                                    w = ECW[ec]
                                    off = 512 * ec
                                    for ti, (lh, rh) in enumerate(
                                            ((pmhi_sb, whi_t), (pmhi_sb, wlo_t),
                                             (pmlo_sb, whi_t))):
                                        nc.tensor.matmul(
                                            psums[qc * 4 + ec][:, :w],
                                            lhsT=lh[:, k, qc * 128:(qc + 1) * 128],
                                            rhs=rh[:, off:off + w],
                                            start=(k == 0 and ti == 0),
                                            stop=(k == 31 and ti == 2))
                        for qc in range(2):
                            for ec in range(4):
                                w = ECW[ec]
                                off = eh * 1792 + 512 * ec
                                nc.scalar.copy(
                                    out=t1t_sb[:, qc, off:off + w],
                                    in_=psums[qc * 4 + ec][:, :w])

                # ------------- phase 1b: L mix + bias ----------------------
                with tc.tile_pool(name='t2p', bufs=1) as t2p, \
                     tc.tile_pool(name='ps2', bufs=4, space='PSUM') as ps2:
                    t2_sb = t2p.tile([128, 28, SP], F32, tag='t2')
                    for et in range(28):
                        ps = ps2.tile([128, SP], F32, tag='mm2')
                        for qc in range(2):
                            nc.tensor.matmul(ps,
                                             lhsT=t1t_sb[:, qc, et * 128:(et + 1) * 128],
                                             rhs=lt_sb[:, qc, :],
                                             start=(qc == 0), stop=(qc == 1))
                        nc.scalar.activation(out=t2_sb[:, et, :], in_=ps,
                                             func=AF.Identity,
                                             bias=bfs_sb[:, et:et + 1])
                    # bounce through DRAM to switch to channel-major layout
                    nc.sync.dma_start(out=t2d.rearrange('(et p) s -> p et s', p=128),
                                      in_=t2_sb)

            # f_sb: channel-major image rows [64, 112*112] (no crop needed)
            with tc.tile_pool(name='pz', bufs=1) as pz:
              stg = pz.tile([64, HALF + M], F32, tag='stg')
              with tc.tile_pool(name='fp', bufs=1) as fp:
                f_sb = fp.tile([64, N], F32, tag='f')
                nc.sync.dma_start(out=f_sb,
                                  in_=t2d.rearrange('(c q) s -> c (q s)', c=64))

                # ------------- phase 1c: BN1 stats + allreduce -------------
                with tc.tile_pool(name='bn1', bufs=1) as bnp1:
                    fv = f_sb.rearrange('c (r w) -> c r w', w=448)
                    recs = bnp1.tile([64, 28, 6], F32, tag='recs')
                    for i in range(28):
                        nc.vector.bn_stats(out=recs[:, i, :], in_=fv[:, i, :])
                    sc = bnp1.tile([64, 24], F32, tag='sc')
                    mv = sc[:, 0:2]
                    tmp = sc[:, 2:3]
                    ssum = sc[:, 3:4]
                    ssq = sc[:, 4:5]
                    su = sc[:, 5:6]
                    qu = sc[:, 6:7]
                    mean = sc[:, 7:8]
                    var = sc[:, 8:9]
                    alpha = sc[:, 9:10]
                    beta = sc[:, 10:11]
                    aqm = sc[:, 11:12]
                    bqm = sc[:, 12:13]
                    azm = sc[:, 13:14]
                    bzm = sc[:, 14:15]
                    stage = sc[:, 16:20]
                    nc.vector.bn_aggr(out=mv, in_=recs)
                    nc.vector.tensor_scalar(out=ssum, in0=mv[:, 0:1], scalar1=float(N),
                                            scalar2=None, op0=AL.mult)
                    nc.vector.tensor_tensor(out=tmp, in0=mv[:, 0:1], in1=mv[:, 0:1],
                                            op=AL.mult)
                    nc.vector.tensor_tensor(out=ssq, in0=mv[:, 1:2], in1=tmp, op=AL.add)
                    nc.vector.tensor_scalar(out=ssq, in0=ssq, scalar1=float(N),
                                            scalar2=None, op0=AL.mult)
                    nc.vector.tensor_scalar(out=stage[:, 0:1], in0=ssum, scalar1=mx64,
                                            scalar2=None, op0=AL.mult)
                    nc.vector.tensor_scalar(out=stage[:, 1:2], in0=ssq, scalar1=mx64,
                                            scalar2=None, op0=AL.mult)
                    nc.vector.tensor_scalar(out=stage[:, 2:3], in0=ssum, scalar1=my64,
                                            scalar2=None, op0=AL.mult)
                    nc.vector.tensor_scalar(out=stage[:, 3:4], in0=ssq, scalar1=my64,
                                            scalar2=None, op0=AL.mult)
                    nc.sync.dma_start(out=cc1_in[:, :], in_=stage)
                    nc.gpsimd.collective_compute(
                        'AllReduce', AL.add, replica_groups=GRP_ALL,
                        ins=[cc1_in.opt()], outs=[cc1_out.opt()])
                    red = bnp1.tile([64, 4], F32, tag='red')
                    nc.sync.dma_start(out=red, in_=cc1_out[:, :])
                    # su/qu: this core's branch stats
                    nc.vector.tensor_scalar(out=tmp, in0=red[:, 2:3], scalar1=my64,
                                            scalar2=None, op0=AL.mult)
                    nc.vector.scalar_tensor_tensor(out=su, in0=red[:, 0:1], scalar=mx64,
                                                   in1=tmp, op0=AL.mult, op1=AL.add)
                    nc.vector.tensor_scalar(out=tmp, in0=red[:, 3:4], scalar1=my64,
                                            scalar2=None, op0=AL.mult)
                    nc.vector.scalar_tensor_tensor(out=qu, in0=red[:, 1:2], scalar=mx64,
                                                   in1=tmp, op0=AL.mult, op1=AL.add)
                    ncnt = 1.0 / (4.0 * N)
                    nc.vector.tensor_scalar(out=mean, in0=su, scalar1=ncnt,
                                            scalar2=None, op0=AL.mult)
                    nc.vector.tensor_scalar(out=var, in0=qu, scalar1=ncnt,
                                            scalar2=None, op0=AL.mult)
                    nc.vector.tensor_tensor(out=tmp, in0=mean, in1=mean, op=AL.mult)
                    nc.vector.tensor_tensor(out=var, in0=var, in1=tmp, op=AL.subtract)
                    nc.vector.tensor_scalar(out=var, in0=var, scalar1=BN_EPS,
                                            scalar2=None, op0=AL.add)
                    nc.scalar.activation(out=var, in_=var, func=AF.Sqrt)
                    nc.vector.reciprocal(out=var, in_=var)
                    nc.vector.tensor_tensor(out=alpha, in0=var, in1=bnp_sb[:, 0:1],
                                            op=AL.mult)
                    nc.vector.tensor_tensor(out=tmp, in0=mean, in1=alpha, op=AL.mult)
                    nc.vector.tensor_tensor(out=beta, in0=bnp_sb[:, 1:2], in1=tmp,
                                            op=AL.subtract)
                    # masked affine params: queries (x role), z (y role)
                    nc.vector.tensor_tensor(out=aqm, in0=alpha, in1=mx64, op=AL.mult)
                    nc.vector.tensor_tensor(out=bqm, in0=beta, in1=mx64, op=AL.mult)
                    nc.vector.tensor_scalar(out=azm, in0=alpha, scalar1=0.25,
                                            scalar2=my64, op0=AL.mult, op1=AL.mult)
                    nc.vector.tensor_tensor(out=bzm, in0=beta, in1=my64, op=AL.mult)

                    # ------------- phase 1d: stage + exchange --------------
                    with tc.tile_pool(name='pzdummy', bufs=1) as _pzd:
                        # bottom-half queries, BN'd, masked by mx
                        nc.vector.tensor_scalar(out=stg[:, 0:HALF],
                                                in0=f_sb[:, HALF:],
                                                scalar1=aqm, scalar2=bqm,
                                                op0=AL.mult, op1=AL.add)
                        # 2x2 sum-pool of raw features (3 strided adds)
                        fv = f_sb.rearrange('c (r a w b) -> c r a w b', r=56, a=2, w=56)
                        zv = stg[:, HALF:].rearrange('c (r w) -> c r w', w=56)
                        nc.vector.tensor_tensor(out=zv, in0=fv[:, :, 0, :, 0],
                                                in1=fv[:, :, 0, :, 1], op=AL.add)
                        nc.vector.tensor_tensor(out=zv, in0=zv,
                                                in1=fv[:, :, 1, :, 0], op=AL.add)
                        nc.vector.tensor_tensor(out=zv, in0=zv,
                                                in1=fv[:, :, 1, :, 1], op=AL.add)
                        nc.vector.tensor_scalar(out=stg[:, HALF:], in0=stg[:, HALF:],
                                                scalar1=azm, scalar2=bzm,
                                                op0=AL.mult, op1=AL.add)
                        nc.sync.dma_start(out=cc2_in[:, :], in_=stg)
                        nc.gpsimd.collective_compute(
                            'AllReduce', AL.add, replica_groups=GRP_PAIR,
                            ins=[cc2_in.opt()], outs=[cc2_out.opt()])
                        # local top half BN'd+masked overwrites the sent staging
                        nc.vector.tensor_scalar(out=stg[:, 0:HALF],
                                                in0=f_sb[:, 0:HALF],
                                                scalar1=aqm, scalar2=bqm,
                                                op0=AL.mult, op1=AL.add)

              # ---------- phase 1e: q, zu, Vt, hi/lo prep (f_sb freed) ----
              with tc.tile_pool(name='pz2', bufs=1) as pz2, \
                     tc.tile_pool(name='psn', bufs=2, space='PSUM') as psn, \
                     tc.tile_pool(name='vtp', bufs=2) as vtp:
                        rq = pz2.tile([64, HALF], F32, tag='rq')
                        tmpf = pz2.tile([64, HALF], F32, tag='tmpf')
                        nc.sync.dma_start(out=rq, in_=cc2_out[:, 0:HALF])
                        z_all = stg[:, HALF:]
                        nc.sync.dma_start(out=z_all, in_=cc2_out[:, HALF:])
                        # q = mx*(local top) + my*(received) in place
                        nc.vector.scalar_tensor_tensor(
                            out=rq, in0=rq, scalar=my64,
                            in1=stg[:, 0:HALF], op0=AL.mult, op1=AL.add)
                        nc.vector.memset(qhi_sb[64:65, :], 1.0)
                        nc.vector.memset(qlo_sb[64:65, :], 0.0)
                        nc.scalar.copy(out=qhi_sb[0:64, :], in_=rq)
                        nc.scalar.copy(out=tmpf, in_=qhi_sb[0:64, :])
                        nc.vector.tensor_tensor(out=qlo_sb[0:64, :],
                                                in0=rq, in1=tmpf,
                                                op=AL.subtract)
                        # zu: normalize z columns
                        zsq = tmpf[:, 0:M]
                        nc.vector.tensor_tensor(out=zsq, in0=z_all, in1=z_all,
                                                op=AL.mult)
                        nsq = pz2.tile([1, M], F32, tag='nsq')
                        for j in range(7):
                            psq = psn.tile([1, 448], F32, tag='nrm')
                            nc.tensor.matmul(psq, lhsT=ones_sb,
                                             rhs=zsq[:, j * 448:(j + 1) * 448],
                                             start=True, stop=True)
                            nc.scalar.copy(out=nsq[:, j * 448:(j + 1) * 448],
                                           in_=psq)
                        nc.vector.reciprocal(out=nsq, in_=nsq)
                        nc.scalar.activation(out=nsq, in_=nsq, func=AF.Sqrt)
                        nc.sync.dma_start(out=rn_d[:, :], in_=nsq)
                        zu = tmpf[:, 0:M]
                        nc.sync.dma_start(out=zu, in_=rn_d[:, :].to_broadcast([64, M]))
                        nc.vector.tensor_tensor(out=zu, in0=z_all, in1=zu,
                                                op=AL.mult)
                        nc.scalar.copy(out=zuhi_sb, in_=zu)
                        zuf = rq[:, 0:M]
                        nc.scalar.copy(out=zuf, in_=zuhi_sb)
                        nc.vector.tensor_tensor(out=zulo_sb, in0=zu, in1=zuf,
                                                op=AL.subtract)
                        # Vt = z^T @ Bw^T  (both cores; identical)
                        for mc in range(25):
                            w = 128 if mc < 24 else 64
                            pv = psn.tile([128, 128], F32, tag='vt')
                            nc.tensor.matmul(pv[:w, :],
                                             lhsT=z_all[:, mc * 128:mc * 128 + w],
                                             rhs=bqt_sb, start=True, stop=True)
                            vt = vtp.tile([128, 128], BF16, tag='vtsb')
                            nc.scalar.copy(out=vt[:w, :], in_=pv[:w, :])
                            nc.sync.dma_start(
                                out=vt_d[mc * 128:mc * 128 + w, :],
                                in_=vt[:w, :])

            if PHASE == '1':
                nc.sync.dma_start(out=out_d[:, :], in_=outpre_sb)

            # ---------------- phase 2: sim + top9 + edgeconv + fc2 ---------
            if PHASE != '1':
                pe_gate(zuhi_sb.bitcast(F32)[:, 0:64])
                with tc.tile_pool(name='psim', bufs=5, space='PSUM') as psim, \
                     tc.tile_pool(name='pmisc', bufs=3, space='PSUM') as pmisc, \
                     tc.tile_pool(name='simp', bufs=2) as simp, \
                     tc.tile_pool(name='selp', bufs=2) as selp, \
                     tc.tile_pool(name='vgp', bufs=2) as vgp, \
                     tc.tile_pool(name='edge', bufs=2) as edgep:
                    for t in range(NT):
                        # --- sim (split-bf16), fp32 PSUM, ACT evict ---
                        sim = simp.tile([128, M], F32, tag='sim')
                        for j in range(7):
                            psj = psim.tile([128, 448], F32, tag='s')
                            for ti, (lh, rh) in enumerate(
                                    ((qhi_sb, zuhi_sb), (qhi_sb, zulo_sb),
                                     (qlo_sb, zuhi_sb))):
                                nc.tensor.matmul(
                                    psj, lhsT=lh[0:64, t * 128:(t + 1) * 128],
                                    rhs=rh[:, j * 448:(j + 1) * 448],
                                    start=(ti == 0), stop=(ti == 2))
                            nc.scalar.copy(out=sim[:, j * 448:(j + 1) * 448],
                                           in_=psj)
                        # --- pack intra-chunk iota into the low byte (DVE) ---
                        simu = sim.bitcast(U32)
                        nc.vector.scalar_tensor_tensor(
                            out=simu, in0=simu, scalar=mask24, in1=iota_sb,
                            op0=AL.bitwise_and, op1=AL.bitwise_or)
                        # --- 8 chunk max8s -> cand64 (transposed [r,k]) ---
                        sel = selp.tile([128, 80], F32, tag='sel')
                        cand = sel[:, 0:64]
                        candt = cand.rearrange('p (r k) -> p r k', k=8)
                        top9v = sel[:, 64:73]
                        pkv = sim.rearrange('p (j k) -> p j k', k=8)
                        for kk in range(8):
                            nc.vector.max(out=candt[:, :, kk], in_=pkv[:, :, kk])
                        # --- merge: top8, replace, 9th ---
                        nc.vector.max(out=top9v[:, 0:8], in_=cand)
                        candz = selp.tile([128, 64], F32, tag='cz')
                        nc.vector.match_replace(out=candz,
                                                in_to_replace=top9v[:, 0:8],
                                                in_values=cand, imm_value=NEG_BIG)
                        nc.vector.tensor_reduce(out=top9v[:, 8:9], in_=candz,
                                                axis=AX.X, op=AL.max)
                        # --- positions: two overlapping FindIndex8 ---
                        idxt = selp.tile([128, 32], F32, tag='idxt')
                        nc.vector.max_index(out=idxt[:, 1:9].bitcast(U32),
                                            in_max=top9v[:, 1:9], in_values=cand)
                        nc.vector.max_index(out=idxt[:, 0:8].bitcast(U32),
                                            in_max=top9v[:, 0:8], in_values=cand)
                        # --- index math: gi = ((val & 0xFF) << 4) | (pos & 15) ---
                        jsh = idxt[:, 16:25].bitcast(U32)
                        gi = selp.tile([128, 9], U32, tag='gi')
                        nc.vector.tensor_scalar(out=jsh, in0=top9v.bitcast(U32),
                                                scalar1=mask8, scalar2=None,
                                                op0=AL.bitwise_and)
                        nc.vector.tensor_scalar(out=jsh, in0=jsh, scalar1=c4u,
                                                scalar2=None,
                                                op0=AL.logical_shift_left)
                        posall = idxt[:, 0:9].bitcast(U32)
                        nc.vector.tensor_scalar(out=gi, in0=posall, scalar1=c15u,
                                                scalar2=None, op0=AL.bitwise_and)
                        nc.vector.tensor_tensor(out=gi, in0=gi, in1=jsh,
                                                op=AL.bitwise_or)
                        # --- gather V rows (9 single-offset indirect DMAs) ---
                        vg = vgp.tile([128, KNN, 128], BF16, tag='vg')
                        if PHASE == '2':
                            nc.vector.memset(vg, 0.0)
                        else:
                            for k in range(KNN):
                                nc.gpsimd.indirect_dma_start(
                                    out=vg[:, k, :], out_offset=None,
                                    in_=vt_d[:, :],
                                    in_offset=bass.IndirectOffsetOnAxis(
                                        ap=gi[:, k:k + 1], axis=0))
                        # --- V-max (DVE bf16 2x; Pool lacks max) ---
                        vmx16 = edgep.tile([128, 256], BF16, tag='vmx16')
                        usbt = edgep.tile([128, 128], F32, tag='usb')
                        va = vmx16[:, 0:128]
                        vb = vmx16[:, 128:256]
                        nc.vector.tensor_tensor(out=va, in0=vg[:, 0, :],
                                                in1=vg[:, 1, :], op=AL.max)
                        nc.vector.tensor_tensor(out=va, in0=va, in1=vg[:, 2, :],
                                                op=AL.max)
                        nc.vector.tensor_tensor(out=va, in0=va, in1=vg[:, 3, :],
                                                op=AL.max)
                        nc.vector.tensor_tensor(out=va, in0=va, in1=vg[:, 4, :],
                                                op=AL.max)
                        nc.vector.tensor_tensor(out=vb, in0=vg[:, 5, :],
                                                in1=vg[:, 6, :], op=AL.max)
                        nc.vector.tensor_tensor(out=vb, in0=vb, in1=vg[:, 7, :],
                                                op=AL.max)
                        nc.vector.tensor_tensor(out=vb, in0=vb, in1=vg[:, 8, :],
                                                op=AL.max)
                        # --- U + vmax ---
                        pu = pmisc.tile([128, 128], F32, tag='m', name=f'pu{t}')
                        nc.tensor.matmul(pu, lhsT=qhi_sb[:, t * 128:(t + 1) * 128],
                                         rhs=ambt_sb, start=True, stop=False)
                        nc.tensor.matmul(pu, lhsT=qlo_sb[:, t * 128:(t + 1) * 128],
                                         rhs=ambt_sb, start=False, stop=True)
                        usb = usbt
                        nc.scalar.copy(out=usb, in_=pu)
                        hts = edgep.tile([128, 256], F32, tag='hts')
                        ht = hts[:, 0:128]
                        hs = hts[:, 128:256]
                        nc.vector.tensor_tensor(out=va, in0=va, in1=vb, op=AL.max)
                        nc.scalar.copy(out=ht, in_=va)
                        nc.gpsimd.tensor_tensor(out=ht, in0=ht, in1=usb, op=AL.add)
                        # transpose, relu on evict, fc2
                        ph = pmisc.tile([128, 128], F32, tag='m', name=f'ph{t}')
                        nc.tensor.transpose(ph, ht, ident_sb)
                        nc.scalar.activation(out=hs, in_=ph, func=AF.Relu)
                        po = pmisc.tile([64, 128], F32, tag='m', name=f'po{t}')
                        nc.tensor.matmul(po, lhsT=fc2wt_sb, rhs=hs,
                                         start=True, stop=True)
                        nc.scalar.activation(out=outpre_sb[:, t * 128:(t + 1) * 128],
                                             in_=po, func=AF.Identity,
                                             bias=bnp_sb[:, 4:5])

                # ------------ phase 3: BN2 + output ------------------------
                with tc.tile_pool(name='bn2', bufs=1) as bnp2:
                    recs2 = bnp2.tile([64, 14, 6], F32, tag='recs2')
                    opv = outpre_sb.rearrange('c (a b) -> c a b', b=448)
                    for i in range(14):
                        nc.vector.bn_stats(out=recs2[:, i, :], in_=opv[:, i, :])
                    sc2 = bnp2.tile([64, 16], F32, tag='sc2')
                    mv2 = sc2[:, 0:2]
                    st2 = sc2[:, 2:4]
                    tmp2 = sc2[:, 4:5]
                    mean2 = sc2[:, 5:6]
                    var2 = sc2[:, 6:7]
                    a2 = sc2[:, 7:8]
                    b2 = sc2[:, 8:9]
                    nc.vector.bn_aggr(out=mv2, in_=recs2)
                    nc.vector.tensor_scalar(out=st2[:, 0:1], in0=mv2[:, 0:1],
                                            scalar1=float(HALF), scalar2=None,
                                            op0=AL.mult)
                    nc.vector.tensor_tensor(out=tmp2, in0=mv2[:, 0:1],
                                            in1=mv2[:, 0:1], op=AL.mult)
                    nc.vector.tensor_tensor(out=st2[:, 1:2], in0=mv2[:, 1:2],
                                            in1=tmp2, op=AL.add)
                    nc.vector.tensor_scalar(out=st2[:, 1:2], in0=st2[:, 1:2],
                                            scalar1=float(HALF), scalar2=None,
                                            op0=AL.mult)
                    nc.sync.dma_start(out=cc3_in[:, :], in_=st2)
                    nc.gpsimd.collective_compute(
                        'AllReduce', AL.add, replica_groups=GRP_ALL,
                        ins=[cc3_in.opt()], outs=[cc3_out.opt()])
                    red2 = bnp2.tile([64, 2], F32, tag='red2')
                    nc.sync.dma_start(out=red2, in_=cc3_out[:, :])
                    ncnt2 = 1.0 / (NCORES * HALF)
                    nc.vector.tensor_scalar(out=mean2, in0=red2[:, 0:1], scalar1=ncnt2,
                                            scalar2=None, op0=AL.mult)
                    nc.vector.tensor_scalar(out=var2, in0=red2[:, 1:2], scalar1=ncnt2,
                                            scalar2=None, op0=AL.mult)
                    nc.vector.tensor_tensor(out=tmp2, in0=mean2, in1=mean2, op=AL.mult)
                    nc.vector.tensor_tensor(out=var2, in0=var2, in1=tmp2,
                                            op=AL.subtract)
                    nc.vector.tensor_scalar(out=var2, in0=var2, scalar1=BN_EPS,
                                            scalar2=None, op0=AL.add)
                    nc.scalar.activation(out=var2, in_=var2, func=AF.Sqrt)
                    nc.vector.reciprocal(out=var2, in_=var2)
                    nc.vector.tensor_tensor(out=a2, in0=var2, in1=bnp_sb[:, 2:3],
                                            op=AL.mult)
                    nc.vector.tensor_tensor(out=tmp2, in0=mean2, in1=a2, op=AL.mult)
                    nc.vector.tensor_tensor(out=b2, in0=bnp_sb[:, 3:4], in1=tmp2,
                                            op=AL.subtract)
                    nc.vector.tensor_scalar(out=outpre_sb, in0=outpre_sb, scalar1=a2,
                                            scalar2=b2, op0=AL.mult, op1=AL.add)
                    nc.sync.dma_start(out=out_d[:, :], in_=outpre_sb)

    nc.compile()
    return nc


_PROGRAM = None


def _get_program():
    global _PROGRAM
    if _PROGRAM is None:
        _PROGRAM = build_program()
    return _PROGRAM


def make_inmaps(inputs):
    prep = _host_prep(inputs)
    x = np.asarray(inputs['x'], np.float32)
    y = np.asarray(inputs['y'], np.float32)
    in_maps = []
    for core in range(NCORES):
        img = x[core] if core < 4 else y[core - 4]
        mx, my = (1.0, 0.0) if core < 4 else (0.0, 1.0)
        masks = np.zeros((128, 2), np.float32)
        masks[:, 0] = mx
        masks[:, 1] = my
        pm = _patchify(img)
        pmhi, pmlo = _split16(pm)
        in_maps.append({
            'pmhi': pmhi, 'pmlo': pmlo,
            'masks': masks,
            **{k: prep[k] for k in ('wfhi', 'wflo', 'ltp', 'bfp', 'ambt', 'bqt',
                                    'fc2wt', 'bnp', 'iota8', 'uconst', 'ident')},
        })
    return in_maps


def assemble(results, inputs):
    x = np.asarray(inputs['x'], np.float32)
    y = np.asarray(inputs['y'], np.float32)
    out = np.empty((4, 64, N), np.float32)
    for b in range(4):
        out[b, :, :HALF] = results[b]['out_half']
        out[b, :, HALF:] = results[b + 4]['out_half']
    out = out.reshape(4, 64, IMG, IMG)
    return out + x, out + y


def kernel(**inputs):
    from concourse.bass_utils import run_bass_kernel_spmd
    nc = _get_program()
    in_maps = make_inmaps(inputs)
    res = run_bass_kernel_spmd(nc, in_maps, core_ids=list(range(NCORES)))
    return assemble(res.results, inputs)
